# revision 24
# baseline (speedup 1.0000x reference)
"""GCN layer (X @ W, then COO spmm scatter-add by dest, + bias) on 8 trn2 cores.

Strategy (dest-sharded, per sharding hint):
  Launch 1 (SPMD): core c computes support shard = X[c*12500:(c+1)*12500] @ W
    in fp16 (fp32 PSUM accumulate). W is the PE-stationary operand; X rows
    stream as N=512 moving tiles into 8 rotating PSUM banks, so the PE runs
    dense and warm. Output is written feature-major (support^T).
  Host: assembles full support; partitions each core's edges by destination;
    greedily packs consecutive dests into groups (<=W_G lanes, <=CAP edge
    slots = 5 tiles of 128); lays the referenced source rows out in edge-slot
    order (the halo-exchange/packing step) plus compact per-slot (lane, val)
    scatter metadata. Host does layout/permutation only - every FLOP (X@W,
    val scaling, segment sum, bias) runs on device.
  Launch 2 (SPMD): per super-op (8 groups): one bulk DMA streams the packed
    G rows; GPSIMD local_scatter expands the (lane, val) metadata into one-hot
    scatter tiles S in SBUF; PE matmul G.T @ S accumulates out^T[128 feats,
    48 dests] per group in PSUM (fusing the val multiply and the segment
    sum); bias added during PSUM evac (fp16 out). DMAs alternate between the
    two HWDGE rings (sync/scalar) to hide per-DMA setup. Host transposes/
    concats shards. Launch 2 runs at the HBM streaming roofline.
"""

import os

import numpy as np

import concourse.bass as bass
import concourse.tile as tile
from concourse import bacc, mybir
from concourse.bass_utils import run_bass_kernel_spmd

try:  # tracing needs antenv.axon_hooks; degrade to no-trace instead of crashing
    import antenv.axon_hooks  # noqa: F401
except Exception:
    if os.environ.get("BASS_TRACE"):
        os.environ["BASS_NEVER_TRACE"] = "1"

# ---------------- problem constants (hardcoded; kernel.py is self-contained)
N_NODES = 100000
N_EDGES = 1600000
IN_F = 256
OUT_F = 128
NCORES = 8

D_PER_CORE = N_NODES // NCORES  # 12500 dest nodes per core

# launch-1 (support matmul) geometry
ROWS_PAD = 12800  # 25 * 512
BLK = 512
NBLK = ROWS_PAD // BLK  # 25
XCHUNK = 8  # input DMA split (rows) so matmuls start early

# launch-2 (stream + spmm) geometry
W_G = 48            # max dest lanes per group
CAP = 640           # edge-slot capacity per group (5 tiles of 128)
TPG = CAP // 128    # tiles per group = 5
SUPER = 8           # groups per super-op (one DMA batch)
TPS = SUPER * TPG   # tiles per super-op = 40

FP32 = mybir.dt.float32
FP16 = mybir.dt.float16
I16 = mybir.dt.int16


def _new_nc():
    return bacc.Bacc("TRN2", target_bir_lowering=False, debug=False)


# ---------------- launch 1: support^T = (X_shard @ W)^T (fp16) --------------
def build_support_program():
    nc = _new_nc()
    xt = nc.declare_dram_parameter("xt", [IN_F, ROWS_PAD], FP16, isOutput=False)
    w = nc.declare_dram_parameter("w", [IN_F, OUT_F], FP16, isOutput=False)
    supT = nc.declare_dram_parameter("supT", [OUT_F, ROWS_PAD], FP16, isOutput=True)

    with tile.TileContext(nc) as tc:
        with (
            tc.tile_pool(name="xt_pool", bufs=1) as xt_pool,
            tc.tile_pool(name="w_pool", bufs=1) as w_pool,
            tc.tile_pool(name="ev_pool", bufs=4) as ev_pool,
            tc.tile_pool(name="ps_pool", bufs=8, space="PSUM") as ps_pool,
        ):
            w_t = w_pool.tile([128, 2, OUT_F], FP16)
            for k in range(2):
                nc.scalar.dma_start(w_t[:, k, :], w[128 * k : 128 * (k + 1), :])
            xt_t = xt_pool.tile([128, 2, ROWS_PAD], FP16)
            cw = ROWS_PAD // XCHUNK
            for c in range(XCHUNK):
                for k in range(2):
                    eng = nc.sync if (2 * c + k) % 2 == 0 else nc.scalar
                    eng.dma_start(
                        xt_t[:, k, c * cw : (c + 1) * cw],
                        xt[128 * k : 128 * (k + 1), c * cw : (c + 1) * cw],
                    )

            for b in range(NBLK):
                ps = ps_pool.tile([128, BLK], FP32, space="PSUM")
                for k in range(2):
                    nc.tensor.matmul(
                        out=ps[:],
                        lhsT=w_t[:, k, :],
                        rhs=xt_t[:, k, BLK * b : BLK * (b + 1)],
                        start=(k == 0),
                        stop=(k == 1),
                    )
                ev = ev_pool.tile([128, BLK], FP16)
                nc.vector.tensor_copy(ev[:], ps[:])
                eng = nc.sync if b % 2 == 0 else nc.scalar
                eng.dma_start(supT[:, BLK * b : BLK * (b + 1)], ev[:])
    nc.compile()
    return nc


# ---------------- launch 2: stream G + on-chip S build + spmm matmul --------
def build_stream_program(ngroups):
    assert ngroups % SUPER == 0
    nsuper = ngroups // SUPER
    nc = _new_nc()
    gs = nc.declare_dram_parameter("gs", [nsuper, 128, TPS, OUT_F], FP16, isOutput=False)
    # smeta[:, :, 0, :] = scatter positions (int16), [:, :, 1, :] = fp16 val bits
    smeta = nc.declare_dram_parameter("smeta", [nsuper, 128, 2, TPS], I16, isOutput=False)
    # super-op 0's S streamed explicitly so its build doesn't wait for the
    # gpsimd library IRAM load (which overlaps super-op 0 instead)
    s0mat = nc.declare_dram_parameter("s0mat", [128, TPS * W_G], FP16, isOutput=False)
    bias = nc.declare_dram_parameter("bias", [OUT_F, 1], FP32, isOutput=False)
    out = nc.declare_dram_parameter("out", [OUT_F, ngroups * W_G], FP16, isOutput=True)

    with tile.TileContext(nc) as tc:
        with (
            tc.tile_pool(name="bias_pool", bufs=1) as bias_pool,
            tc.tile_pool(name="g_pool", bufs=6) as g_pool,
            tc.tile_pool(name="m_pool", bufs=3) as m_pool,
            tc.tile_pool(name="s_pool", bufs=3) as s_pool,
            tc.tile_pool(name="stage_pool", bufs=3) as stage_pool,
            tc.tile_pool(name="ps_pool", bufs=8, space="PSUM") as ps_pool,
        ):
            bias_t = bias_pool.tile([128, 1], FP32)
            nc.sync.dma_start(bias_t[:], bias[:, :])

            for s in range(nsuper):
                g_t = g_pool.tile([128, TPS, OUT_F], FP16)
                eng = nc.sync if s % 2 == 0 else nc.scalar
                eng2 = nc.scalar if s % 2 == 0 else nc.sync
                h = TPS // 2
                eng.dma_start(g_t[:, :h, :], gs[s][:, :h, :])
                eng2.dma_start(g_t[:, h:, :], gs[s][:, h:, :])
                s_t = s_pool.tile([128, TPS * W_G], FP16)
                if s == 0:
                    eng2.dma_start(s_t[:], s0mat[:, :])
                else:
                    m_t = m_pool.tile([128, 2, TPS], I16)
                    eng2.dma_start(m_t[:], smeta[s])
                    nc.gpsimd.local_scatter(
                        s_t[:], m_t[:, 1, :].bitcast(FP16), m_t[:, 0, :], 128,
                        TPS * W_G, TPS,
                    )

                stage = stage_pool.tile([128, SUPER * W_G], FP16)
                for gi in range(SUPER):
                    ps = ps_pool.tile([128, W_G], FP32, space="PSUM")
                    for t in range(TPG):
                        k = gi * TPG + t
                        nc.tensor.matmul(
                            out=ps[:],
                            lhsT=g_t[:, k, :],
                            rhs=s_t[:, W_G * k : W_G * (k + 1)],
                            start=(t == 0),
                            stop=(t == TPG - 1),
                        )
                    nc.vector.tensor_scalar(
                        out=stage[:, W_G * gi : W_G * (gi + 1)],
                        in0=ps[:],
                        scalar1=bias_t[:],
                        scalar2=None,
                        op0=mybir.AluOpType.add,
                    )
                eng.dma_start(
                    out[:, SUPER * W_G * s : SUPER * W_G * (s + 1)], stage[:]
                )
    nc.compile()
    return nc


# ---------------- host-side packing ----------------
def _pack_core_meta(rows_c):
    """Greedy group packing for one core's dest-sorted edges.

    rows_c: local dest ids [0, D_PER_CORE). Returns per-edge (slot, lane),
    per-dest (gid, lane) and the group count.
    """
    cnt = np.bincount(rows_c, minlength=D_PER_CORE).astype(np.int64)
    assert cnt.max() <= CAP, f"dest degree {cnt.max()} exceeds CAP {CAP}"
    gid = np.empty(D_PER_CORE, np.int64)
    lane = np.empty(D_PER_CORE, np.int64)
    g = 0
    e = 0
    l = 0
    for d in range(D_PER_CORE):
        c = cnt[d]
        if e + c > CAP or l >= W_G:
            g += 1
            e = 0
            l = 0
        gid[d] = g
        lane[d] = l
        l += 1
        e += c
    ngroups = g + 1

    cs = np.cumsum(cnt) - cnt  # global (dest-sorted) edge prefix per dest
    first_d = np.unique(gid, return_index=True)[1]  # first dest of each group
    within_group_prefix = cs - cs[first_d[gid]]
    dest_slot_start = gid * CAP + within_group_prefix

    order = np.argsort(rows_c, kind="stable")
    r_s = rows_c[order]
    within_dest = np.arange(len(r_s), dtype=np.int64) - cs[r_s]
    slot = dest_slot_start[r_s] + within_dest
    lane_e = lane[r_s]
    return order, slot, lane_e, gid, lane, ngroups


def _pack_core_arrays(cols_s, vals_s, slot, lane_e, ngroups, support_f16):
    """Build (gs, sval, sidx) stream arrays for one core."""
    nslots = ngroups * CAP
    ntiles = nslots // 128
    nsuper = ngroups // SUPER

    g_lin = np.zeros((nslots, OUT_F), np.float16)
    g_lin[slot] = support_f16[cols_s]
    gs = np.ascontiguousarray(
        g_lin.reshape(nsuper, TPS, 128, OUT_F).transpose(0, 2, 1, 3)
    )
    del g_lin

    tile_of = slot // 128  # global tile index
    p_of = slot % 128
    k_of = tile_of % TPS  # tile within super-op

    sval = np.zeros((ntiles, 128), np.float16)
    sval[tile_of, p_of] = vals_s.astype(np.float16)
    sidx = np.full((ntiles, 128), -1, np.int16)
    sidx[tile_of, p_of] = (k_of * W_G + lane_e).astype(np.int16)
    smeta = np.stack(
        [
            sidx.reshape(nsuper, TPS, 128),
            sval.view(np.int16).reshape(nsuper, TPS, 128),
        ],
        axis=2,
    )  # [nsuper, TPS, 2, 128]
    smeta = np.ascontiguousarray(smeta.transpose(0, 3, 2, 1))

    s0mat = np.zeros((128, TPS, W_G), np.float16)
    m0 = tile_of < TPS
    s0mat[p_of[m0], tile_of[m0], lane_e[m0]] = vals_s[m0].astype(np.float16)
    s0mat = s0mat.reshape(128, TPS * W_G)
    return gs, smeta, s0mat


def kernel(X_input, adj_row, adj_col, adj_val, W, bias):
    X_input = np.asarray(X_input, np.float32)
    adj_row = np.asarray(adj_row)
    adj_col = np.asarray(adj_col)
    adj_val = np.asarray(adj_val, np.float32)
    W = np.asarray(W, np.float32)
    bias = np.asarray(bias, np.float32)

    # ---- launch 1: support shards (fp16, transposed out)
    nc1 = build_support_program()
    xT = np.ascontiguousarray(X_input.T.astype(np.float16))
    w16 = W.astype(np.float16)
    in_maps1 = []
    for c in range(NCORES):
        sl = np.zeros((IN_F, ROWS_PAD), np.float16)
        lo = c * D_PER_CORE
        sl[:, :D_PER_CORE] = xT[:, lo : lo + D_PER_CORE]
        in_maps1.append({"xt": sl, "w": w16})
    res1 = run_bass_kernel_spmd(nc1, in_maps1, list(range(NCORES)))
    kernel.last_res1 = res1
    support_f16 = np.ascontiguousarray(
        np.concatenate(
            [res1.results[c]["supT"][:, :D_PER_CORE] for c in range(NCORES)], axis=1
        ).T
    )  # [100000, 128] fp16

    # ---- host packing: per-core greedy groups + slot-order stream layout
    core_of = adj_row // D_PER_CORE
    metas = []
    for c in range(NCORES):
        m = core_of == c
        rows_c = (adj_row[m] - c * D_PER_CORE).astype(np.int64)
        cols_c = adj_col[m].astype(np.int64)
        vals_c = adj_val[m]
        order, slot, lane_e, gid, lane, ngroups = _pack_core_meta(rows_c)
        metas.append((cols_c[order], vals_c[order], slot, lane_e, gid, lane, ngroups))
    ngroups_all = max(m[6] for m in metas)
    NGROUPS = -(-ngroups_all // SUPER) * SUPER  # round up to SUPER

    in_maps2 = []
    bias_col = np.ascontiguousarray(bias.reshape(OUT_F, 1))
    for c in range(NCORES):
        cols_s, vals_s, slot, lane_e, gid, lane, _ = metas[c]
        gs, smeta, s0mat = _pack_core_arrays(
            cols_s, vals_s, slot, lane_e, NGROUPS, support_f16
        )
        in_maps2.append(
            {"gs": gs, "smeta": smeta, "s0mat": s0mat, "bias": bias_col}
        )

    # ---- launch 2
    nc2 = build_stream_program(NGROUPS)
    res2 = run_bass_kernel_spmd(nc2, in_maps2, list(range(NCORES)))
    kernel.last_res2 = res2
    out = np.empty((N_NODES, OUT_F), np.float32)
    for c in range(NCORES):
        o = res2.results[c]["out"].astype(np.float32)  # [128, NGROUPS*W_G]
        gid, lane = metas[c][4], metas[c][5]
        colidx = gid * W_G + lane
        out[c * D_PER_CORE : (c + 1) * D_PER_CORE] = o[:, colidx].T
    return out


# revision 26
# speedup vs baseline: 1.0468x; 1.0468x over previous
"""GCN layer (X @ W, then COO spmm scatter-add by dest, + bias) on 8 trn2 cores.

Strategy (dest-sharded, per sharding hint):
  Launch 1 (SPMD): core c computes support shard = X[c*12500:(c+1)*12500] @ W
    in fp16 (fp32 PSUM accumulate). W is the PE-stationary operand; X rows
    stream as N=512 moving tiles into 8 rotating PSUM banks, so the PE runs
    dense and warm. Output is written feature-major (support^T).
  Host: assembles full support; partitions each core's edges by destination;
    greedily packs consecutive dests into groups (<=W_G lanes, <=CAP edge
    slots = 5 tiles of 128); lays the referenced source rows out in edge-slot
    order (the halo-exchange/packing step) plus compact per-slot (lane, val)
    scatter metadata. Host does layout/permutation only - every FLOP (X@W,
    val scaling, segment sum, bias) runs on device.
  Launch 2 (SPMD): per super-op (8 groups): one bulk DMA streams the packed
    G rows; GPSIMD local_scatter expands the (lane, val) metadata into one-hot
    scatter tiles S in SBUF; PE matmul G.T @ S accumulates out^T[128 feats,
    48 dests] per group in PSUM (fusing the val multiply and the segment
    sum); bias added during PSUM evac (fp16 out). DMAs alternate between the
    two HWDGE rings (sync/scalar) to hide per-DMA setup. Host transposes/
    concats shards. Launch 2 runs at the HBM streaming roofline.
"""

import os

import numpy as np

import concourse.bass as bass
import concourse.tile as tile
from concourse import bacc, mybir
from concourse.bass_utils import run_bass_kernel_spmd

try:  # tracing needs antenv.axon_hooks; degrade to no-trace instead of crashing
    import antenv.axon_hooks  # noqa: F401
except Exception:
    if os.environ.get("BASS_TRACE"):
        os.environ["BASS_NEVER_TRACE"] = "1"

# ---------------- problem constants (hardcoded; kernel.py is self-contained)
N_NODES = 100000
N_EDGES = 1600000
IN_F = 256
OUT_F = 128
NCORES = 8

D_PER_CORE = N_NODES // NCORES  # 12500 dest nodes per core

# launch-1 (support matmul) geometry
ROWS_PAD = 12800  # 25 * 512
BLK = 512
NBLK = ROWS_PAD // BLK  # 25
XCHUNK = 8  # input DMA split (rows) so matmuls start early

# launch-2 (stream + spmm) geometry
W_G = 48            # max dest lanes per group
CAP = 640           # edge-slot capacity per group (5 tiles of 128)
TPG = CAP // 128    # tiles per group = 5
SUPER = 8           # groups per super-op (one DMA batch)
TPS = SUPER * TPG   # tiles per super-op = 40

FP32 = mybir.dt.float32
FP16 = mybir.dt.float16
I16 = mybir.dt.int16


def _new_nc():
    return bacc.Bacc("TRN2", target_bir_lowering=False, debug=False)


# ---------------- launch 1: support^T = (X_shard @ W)^T (fp16) --------------
def build_support_program():
    nc = _new_nc()
    xt = nc.declare_dram_parameter("xt", [IN_F, ROWS_PAD], FP16, isOutput=False)
    w = nc.declare_dram_parameter("w", [IN_F, OUT_F], FP16, isOutput=False)
    supT = nc.declare_dram_parameter("supT", [OUT_F, ROWS_PAD], FP16, isOutput=True)

    with tile.TileContext(nc) as tc:
        with (
            tc.tile_pool(name="xt_pool", bufs=1) as xt_pool,
            tc.tile_pool(name="w_pool", bufs=1) as w_pool,
            tc.tile_pool(name="ev_pool", bufs=4) as ev_pool,
            tc.tile_pool(name="ps_pool", bufs=8, space="PSUM") as ps_pool,
        ):
            w_t = w_pool.tile([128, 2, OUT_F], FP16)
            for k in range(2):
                nc.scalar.dma_start(w_t[:, k, :], w[128 * k : 128 * (k + 1), :])
            xt_t = xt_pool.tile([128, 2, ROWS_PAD], FP16)
            cw = ROWS_PAD // XCHUNK
            for c in range(XCHUNK):
                for k in range(2):
                    eng = nc.sync if (2 * c + k) % 2 == 0 else nc.scalar
                    eng.dma_start(
                        xt_t[:, k, c * cw : (c + 1) * cw],
                        xt[128 * k : 128 * (k + 1), c * cw : (c + 1) * cw],
                    )

            b0 = 0
            while b0 < NBLK:
                nb = min(8, NBLK - b0)
                pss = [
                    ps_pool.tile([128, BLK], FP32, space="PSUM", name="ps", tag="ps")
                    for _ in range(nb)
                ]
                for k in range(2):
                    for j in range(nb):
                        b = b0 + j
                        nc.tensor.matmul(
                            out=pss[j][:],
                            lhsT=w_t[:, k, :],
                            rhs=xt_t[:, k, BLK * b : BLK * (b + 1)],
                            start=(k == 0),
                            stop=(k == 1),
                        )
                for j in range(nb):
                    b = b0 + j
                    ev = ev_pool.tile([128, BLK], FP16)
                    nc.vector.tensor_copy(ev[:], pss[j][:])
                    eng = nc.sync if b % 2 == 0 else nc.scalar
                    eng.dma_start(supT[:, BLK * b : BLK * (b + 1)], ev[:])
                b0 += nb
    nc.compile()
    return nc


# ---------------- launch 2: stream G + on-chip S build + spmm matmul --------
def build_stream_program(ngroups):
    assert ngroups % SUPER == 0
    nsuper = ngroups // SUPER
    nc = _new_nc()
    gs = nc.declare_dram_parameter("gs", [nsuper, 128, TPS, OUT_F], FP16, isOutput=False)
    # smeta[:, :, 0, :] = scatter positions (int16), [:, :, 1, :] = fp16 val bits
    smeta = nc.declare_dram_parameter("smeta", [nsuper, 128, 2, TPS], I16, isOutput=False)
    # super-op 0's S streamed explicitly so its build doesn't wait for the
    # gpsimd library IRAM load (which overlaps super-op 0 instead)
    s0mat = nc.declare_dram_parameter("s0mat", [128, TPS * W_G], FP16, isOutput=False)
    bias = nc.declare_dram_parameter("bias", [OUT_F, 1], FP32, isOutput=False)
    out = nc.declare_dram_parameter("out", [OUT_F, ngroups * W_G], FP16, isOutput=True)

    with tile.TileContext(nc) as tc:
        with (
            tc.tile_pool(name="bias_pool", bufs=1) as bias_pool,
            tc.tile_pool(name="g_pool", bufs=6) as g_pool,
            tc.tile_pool(name="m_pool", bufs=3) as m_pool,
            tc.tile_pool(name="s_pool", bufs=3) as s_pool,
            tc.tile_pool(name="stage_pool", bufs=3) as stage_pool,
            tc.tile_pool(name="ps_pool", bufs=8, space="PSUM") as ps_pool,
        ):
            bias_t = bias_pool.tile([128, 1], FP32)
            nc.sync.dma_start(bias_t[:], bias[:, :])

            for s in range(nsuper):
                g_t = g_pool.tile([128, TPS, OUT_F], FP16)
                eng = nc.sync if s % 2 == 0 else nc.scalar
                eng2 = nc.scalar if s % 2 == 0 else nc.sync
                h = TPS // 2
                s_t = s_pool.tile([128, TPS * W_G], FP16)
                if s == 0:
                    # S first on eng2: the first MMs need g-half1 + S only,
                    # so they don't queue behind g-half2 in eng2's FIFO
                    eng2.dma_start(s_t[:], s0mat[:, :])
                eng.dma_start(g_t[:, :h, :], gs[s][:, :h, :])
                eng2.dma_start(g_t[:, h:, :], gs[s][:, h:, :])
                if s != 0:
                    m_t = m_pool.tile([128, 2, TPS], I16)
                    eng2.dma_start(m_t[:], smeta[s])
                    nc.gpsimd.local_scatter(
                        s_t[:], m_t[:, 1, :].bitcast(FP16), m_t[:, 0, :], 128,
                        TPS * W_G, TPS,
                    )

                stage = stage_pool.tile([128, SUPER * W_G], FP16)
                for gi in range(SUPER):
                    ps = ps_pool.tile([128, W_G], FP32, space="PSUM")
                    for t in range(TPG):
                        k = gi * TPG + t
                        nc.tensor.matmul(
                            out=ps[:],
                            lhsT=g_t[:, k, :],
                            rhs=s_t[:, W_G * k : W_G * (k + 1)],
                            start=(t == 0),
                            stop=(t == TPG - 1),
                        )
                    nc.vector.tensor_scalar(
                        out=stage[:, W_G * gi : W_G * (gi + 1)],
                        in0=ps[:],
                        scalar1=bias_t[:],
                        scalar2=None,
                        op0=mybir.AluOpType.add,
                    )
                eng.dma_start(
                    out[:, SUPER * W_G * s : SUPER * W_G * (s + 1)], stage[:]
                )
    nc.compile()
    return nc


# ---------------- host-side packing ----------------
def _pack_core_meta(rows_c):
    """Greedy group packing for one core's dest-sorted edges.

    rows_c: local dest ids [0, D_PER_CORE). Returns per-edge (slot, lane),
    per-dest (gid, lane) and the group count.
    """
    cnt = np.bincount(rows_c, minlength=D_PER_CORE).astype(np.int64)
    assert cnt.max() <= CAP, f"dest degree {cnt.max()} exceeds CAP {CAP}"
    gid = np.empty(D_PER_CORE, np.int64)
    lane = np.empty(D_PER_CORE, np.int64)
    g = 0
    e = 0
    l = 0
    for d in range(D_PER_CORE):
        c = cnt[d]
        if e + c > CAP or l >= W_G:
            g += 1
            e = 0
            l = 0
        gid[d] = g
        lane[d] = l
        l += 1
        e += c
    ngroups = g + 1

    cs = np.cumsum(cnt) - cnt  # global (dest-sorted) edge prefix per dest
    first_d = np.unique(gid, return_index=True)[1]  # first dest of each group
    within_group_prefix = cs - cs[first_d[gid]]
    dest_slot_start = gid * CAP + within_group_prefix

    order = np.argsort(rows_c, kind="stable")
    r_s = rows_c[order]
    within_dest = np.arange(len(r_s), dtype=np.int64) - cs[r_s]
    slot = dest_slot_start[r_s] + within_dest
    lane_e = lane[r_s]
    return order, slot, lane_e, gid, lane, ngroups


def _pack_core_arrays(cols_s, vals_s, slot, lane_e, ngroups, support_f16):
    """Build (gs, sval, sidx) stream arrays for one core."""
    nslots = ngroups * CAP
    ntiles = nslots // 128
    nsuper = ngroups // SUPER

    g_lin = np.zeros((nslots, OUT_F), np.float16)
    g_lin[slot] = support_f16[cols_s]
    gs = np.ascontiguousarray(
        g_lin.reshape(nsuper, TPS, 128, OUT_F).transpose(0, 2, 1, 3)
    )
    del g_lin

    tile_of = slot // 128  # global tile index
    p_of = slot % 128
    k_of = tile_of % TPS  # tile within super-op

    sval = np.zeros((ntiles, 128), np.float16)
    sval[tile_of, p_of] = vals_s.astype(np.float16)
    sidx = np.full((ntiles, 128), -1, np.int16)
    sidx[tile_of, p_of] = (k_of * W_G + lane_e).astype(np.int16)
    smeta = np.stack(
        [
            sidx.reshape(nsuper, TPS, 128),
            sval.view(np.int16).reshape(nsuper, TPS, 128),
        ],
        axis=2,
    )  # [nsuper, TPS, 2, 128]
    smeta = np.ascontiguousarray(smeta.transpose(0, 3, 2, 1))

    s0mat = np.zeros((128, TPS, W_G), np.float16)
    m0 = tile_of < TPS
    s0mat[p_of[m0], tile_of[m0], lane_e[m0]] = vals_s[m0].astype(np.float16)
    s0mat = s0mat.reshape(128, TPS * W_G)
    return gs, smeta, s0mat


def kernel(X_input, adj_row, adj_col, adj_val, W, bias):
    X_input = np.asarray(X_input, np.float32)
    adj_row = np.asarray(adj_row)
    adj_col = np.asarray(adj_col)
    adj_val = np.asarray(adj_val, np.float32)
    W = np.asarray(W, np.float32)
    bias = np.asarray(bias, np.float32)

    # ---- launch 1: support shards (fp16, transposed out)
    nc1 = build_support_program()
    xT = np.ascontiguousarray(X_input.T.astype(np.float16))
    w16 = W.astype(np.float16)
    in_maps1 = []
    for c in range(NCORES):
        sl = np.zeros((IN_F, ROWS_PAD), np.float16)
        lo = c * D_PER_CORE
        sl[:, :D_PER_CORE] = xT[:, lo : lo + D_PER_CORE]
        in_maps1.append({"xt": sl, "w": w16})
    res1 = run_bass_kernel_spmd(nc1, in_maps1, list(range(NCORES)))
    kernel.last_res1 = res1
    support_f16 = np.ascontiguousarray(
        np.concatenate(
            [res1.results[c]["supT"][:, :D_PER_CORE] for c in range(NCORES)], axis=1
        ).T
    )  # [100000, 128] fp16

    # ---- host packing: per-core greedy groups + slot-order stream layout
    core_of = adj_row // D_PER_CORE
    metas = []
    for c in range(NCORES):
        m = core_of == c
        rows_c = (adj_row[m] - c * D_PER_CORE).astype(np.int64)
        cols_c = adj_col[m].astype(np.int64)
        vals_c = adj_val[m]
        order, slot, lane_e, gid, lane, ngroups = _pack_core_meta(rows_c)
        metas.append((cols_c[order], vals_c[order], slot, lane_e, gid, lane, ngroups))
    ngroups_all = max(m[6] for m in metas)
    NGROUPS = -(-ngroups_all // SUPER) * SUPER  # round up to SUPER

    in_maps2 = []
    bias_col = np.ascontiguousarray(bias.reshape(OUT_F, 1))
    for c in range(NCORES):
        cols_s, vals_s, slot, lane_e, gid, lane, _ = metas[c]
        gs, smeta, s0mat = _pack_core_arrays(
            cols_s, vals_s, slot, lane_e, NGROUPS, support_f16
        )
        in_maps2.append(
            {"gs": gs, "smeta": smeta, "s0mat": s0mat, "bias": bias_col}
        )

    # ---- launch 2
    nc2 = build_stream_program(NGROUPS)
    res2 = run_bass_kernel_spmd(nc2, in_maps2, list(range(NCORES)))
    kernel.last_res2 = res2
    out = np.empty((N_NODES, OUT_F), np.float32)
    for c in range(NCORES):
        o = res2.results[c]["out"].astype(np.float32)  # [128, NGROUPS*W_G]
        gid, lane = metas[c][4], metas[c][5]
        colidx = gid * W_G + lane
        out[c * D_PER_CORE : (c + 1) * D_PER_CORE] = o[:, colidx].T
    return out
